# revision 36
# baseline (speedup 1.0000x reference)
"""GCContext (global-context pooling) Trainium2 Bass kernel.

Problem (per sample): x [C=1024, HW=4096] fp32
  logits = (w @ x + b) / sqrt(C)        # [HW]
  attn   = softmax(logits)              # [HW]
  focus  = x @ attn                     # [C]
Output: [B, C, 1, 1].

Design (B=16 data-parallel over 8 cores, 2 samples/core, fp16 data path):
  - x and w are cast to fp16 on the host (halves HBM traffic; output error
    ~2e-4 relative). attn, Z, and all accumulations stay fp32.
  - x is streamed in 2MB pieces of 1024 spatial positions, host-permuted to
    [b, piece, partition, half, chunk, s]; the two 1MB halves of each piece
    go to the two HWDGE rings (sync/scalar) so pieces complete in order and
    matmuls start after the first 1MB.
  - PE computes logits with a replicated-w stationary [128,128], so each
    PSUM bank holds 512 logits broadcast across all 128 partitions - the
    partition-broadcast layout the vector engine needs for the focus
    multiply (no transposes or cross-partition reductions anywhere).
  - ACT does exp((logits+b)/32) from PSUM with fused per-partition
    Z-accumulation (accum_out). No max-subtraction: logits are ~N(0, 0.02^2)
    by construction so exp is safe, and softmax is shift-invariant.
  - DVE does the focus contraction with fused scalar_tensor_tensor
    (x*attn multiplied and summed along the free dim into accum_out), one
    FD=1024 op per channel chunk per piece, partial sums reduced at the
    end; first/last pieces run at FD=512 half-piece granularity to shorten
    the pipeline fill and drain chains. DVE is the bottleneck (~80us busy,
    ~97% occupancy; DMA ~60us, PE ~63us hidden underneath).
  - Final normalize by 1/Z and store [128, 8] per sample; host reassembles.
"""

import sys

for _p in ("/opt/trn_rl_repo",):
    if _p not in sys.path:
        sys.path.insert(0, _p)

import numpy as np

import concourse.bacc as bacc
import concourse.tile as tile
from concourse import mybir
from concourse.bass_utils import run_bass_kernel_spmd

N_CORES = 8
B = 16
C = 1024
H = 64
W = 64
HW = H * W
B_LOC = B // N_CORES          # samples per core
R = C // 128                  # channel chunks (partition groups)
NS = 8                        # spatial slices per sample
S = HW // NS                  # spatial positions per slice (512)
SCALE = 1.0 / 32.0            # 1/sqrt(C)

_CACHE = {}


def _build_nc():
    nc = bacc.Bacc("TRN2", target_bir_lowering=False, debug=False,
                   num_devices=N_CORES)
    fp32 = mybir.dt.float32

    fp16 = mybir.dt.float16
    xs = nc.dram_tensor("xs", [B_LOC, NS // 2, 128, 2, R, S], fp16,
                        kind="ExternalInput")
    wrep = nc.dram_tensor("wrep", [128, R, 128], fp16, kind="ExternalInput")
    bias = nc.dram_tensor("bias", [128, 1], fp32, kind="ExternalInput")
    out = nc.dram_tensor("focus_out", [B_LOC, 128, R], fp32, kind="ExternalOutput")

    with tile.TileContext(nc) as tc:
        with (
            tc.tile_pool(name="singles", bufs=1) as singles,
            tc.tile_pool(name="xp", bufs=4) as xp,
            tc.tile_pool(name="attnp", bufs=2) as attnp,
            tc.tile_pool(name="accp", bufs=4) as accp,
            tc.tile_pool(name="scrp", bufs=2) as scrp,
            tc.tile_pool(name="smallp", bufs=4) as smallp,
            tc.tile_pool(name="psum", bufs=4, space="PSUM") as psump,
        ):
            w_sb = singles.tile([128, R, 128], fp16)
            nc.scalar.dma_start(out=w_sb[:], in_=wrep[:])
            bias_sb = singles.tile([128, 1], fp32)
            nc.scalar.dma_start(out=bias_sb[:], in_=bias[:])

            NJ = NS // 2
            for b in range(B_LOC):
                attn_t = attnp.tile([128, NS, S], fp32)
                # NJ piece slots + 2 spare slots for the k-granular
                # head/tail pieces (their unused slots stay zero)
                fparts = accp.tile([128, R, NJ + 2], fp32)
                nc.vector.memset(fparts[:], 0.0)
                zpart = accp.tile([128, NS], fp32)
                for j in range(NJ):
                    x_t = xp.tile([128, 2, R, S], fp16)
                    # halves go to alternating HWDGE rings so pieces complete
                    # in order and matmuls start after the first 1MB
                    nc.sync.dma_start(out=x_t[:, 0], in_=xs[b, j, :, 0])
                    nc.scalar.dma_start(out=x_t[:, 1], in_=xs[b, j, :, 1])
                    head = (b == 0 and j == 0)
                    tail = (b == B_LOC - 1 and j == NJ - 1)
                    ps = [psump.tile([128, S], fp32, name=f"ps{k}",
                                     tag=f"ps{k}")
                          for k in range(2)]
                    if head:
                        # k-outer: bank 0 completes without waiting for the
                        # second DMA half (shortens pipeline fill)
                        for k in range(2):
                            for r in range(R):
                                nc.tensor.matmul(
                                    ps[k][:],
                                    lhsT=w_sb[:, r, :],
                                    rhs=x_t[:, k, r, :],
                                    start=(r == 0), stop=(r == R - 1))
                            nc.scalar.activation(
                                attn_t[:, 2 * j + k, :], ps[k][:],
                                mybir.ActivationFunctionType.Exp,
                                bias=bias_sb[:, 0:1], scale=SCALE,
                                accum_out=zpart[:, 2 * j + k:2 * j + k + 1])
                    else:
                        # r-outer: one stationary load per channel chunk
                        for r in range(R):
                            for k in range(2):
                                nc.tensor.matmul(
                                    ps[k][:],
                                    lhsT=w_sb[:, r, :],
                                    rhs=x_t[:, k, r, :],
                                    start=(r == 0), stop=(r == R - 1))
                        for k in range(2):
                            h = 2 * j + k
                            nc.scalar.activation(
                                attn_t[:, h, :], ps[k][:],
                                mybir.ActivationFunctionType.Exp,
                                bias=bias_sb[:, 0:1], scale=SCALE,
                                accum_out=zpart[:, h:h + 1])
                    if not (tail or head):
                        for r in range(R):
                            scr = scrp.tile([128, 2, S], fp32,
                                            name=f"scr{r % 2}",
                                            tag=f"scr{r % 2}")
                            nc.vector.scalar_tensor_tensor(
                                out=scr[:],
                                in0=x_t[:, :, r, :],
                                scalar=1.0,
                                in1=attn_t[:, 2 * j:2 * j + 2, :],
                                op0=mybir.AluOpType.mult,
                                op1=mybir.AluOpType.mult,
                                accum_out=fparts[:, r, j:j + 1])
                    else:
                        # first/last piece: k-granular so fill and tail
                        # chains are one bank of MMs + exp + FD=512 focus
                        # ops; halves accumulate into separate fparts slots
                        for k in range(2):
                            slot = j if k == 0 else NJ + (0 if head else 1)
                            for r in range(R):
                                scr2 = scrp.tile([128, 2, S], fp32)
                                nc.vector.scalar_tensor_tensor(
                                    out=scr2[:, 0, :],
                                    in0=x_t[:, k, r, :],
                                    scalar=1.0,
                                    in1=attn_t[:, 2 * j + k, :],
                                    op0=mybir.AluOpType.mult,
                                    op1=mybir.AluOpType.mult,
                                    accum_out=fparts[:, r, slot:slot + 1])
                ztot = smallp.tile([128, 1], fp32)
                nc.vector.tensor_reduce(ztot[:], zpart[:],
                                        axis=mybir.AxisListType.X,
                                        op=mybir.AluOpType.add)
                rz = smallp.tile([128, 1], fp32)
                nc.vector.reciprocal(rz[:], ztot[:])
                facc = smallp.tile([128, R], fp32)
                nc.vector.tensor_reduce(facc[:], fparts[:],
                                        axis=mybir.AxisListType.X,
                                        op=mybir.AluOpType.add)
                fout = smallp.tile([128, R], fp32)
                nc.vector.tensor_scalar_mul(fout[:], facc[:], rz[:, 0:1])
                nc.sync.dma_start(out=out[b], in_=fout[:])

    nc.compile()
    return nc


def _get_nc():
    if "nc" not in _CACHE:
        _CACHE["nc"] = _build_nc()
    return _CACHE["nc"]


def _prep_core_inputs(x, key_w, key_b):
    """Build the per-core input maps (host-side shard + layout permute)."""
    # [B, C, H, W] -> [B, R, 128, NS/2, 2, S] -> [B, NS/2, 128, 2, R, S]
    xv = np.ascontiguousarray(
        x.reshape(B, R, 128, NS // 2, 2, S).transpose(0, 3, 2, 4, 1, 5)
    ).astype(np.float16)
    wrep = np.ascontiguousarray(
        np.broadcast_to(key_w.reshape(R, 128).T[:, :, None], (128, R, 128))
    ).astype(np.float16)
    bias = np.full((128, 1), key_b[0] * SCALE, dtype=np.float32)
    in_maps = []
    for c in range(N_CORES):
        in_maps.append({
            "xs": xv[c * B_LOC:(c + 1) * B_LOC],
            "wrep": wrep,
            "bias": bias,
        })
    return in_maps


def kernel(x, key_w, key_b):
    x = np.asarray(x, dtype=np.float32)
    key_w = np.asarray(key_w, dtype=np.float32)
    key_b = np.asarray(key_b, dtype=np.float32)
    assert x.shape == (B, C, H, W), x.shape

    nc = _get_nc()
    in_maps = _prep_core_inputs(x, key_w, key_b)
    res = run_bass_kernel_spmd(nc, in_maps, list(range(N_CORES)))

    out = np.empty((B, C), dtype=np.float32)
    for c in range(N_CORES):
        f = res.results[c]["focus_out"]          # [B_LOC, 128, R]
        out[c * B_LOC:(c + 1) * B_LOC] = (
            f.transpose(0, 2, 1).reshape(B_LOC, C))
    return out.reshape(B, C, 1, 1)
